# revision 41
# baseline (speedup 1.0000x reference)
"""DeepSet kernel for Trainium2 (8 NeuronCores, data-parallel).

Model (reference):
    mask  = sign(|sum_e words|)                  # padding rows are all-zero
    h1    = tanh(words @ W1 + b1)                # [B,S,H]
    h2    = tanh(h1 @ W2 + b2)                   # [B,S,H]
    enc   = h2 @ W3 + b3                         # [B,S,C]
    codes = sum_s enc * mask                     # [B,C]
    out   = (tanh(tanh(codes@W4+b4)@W5+b5)) @ W6 + b6   # [B,T]

Algebraic restructuring: codes = (sum_s mask*h2) @ W3 + N_b * b3, so only the
two big MLP layers run on device; the tiny decode runs on host.

Layout strategy (all bf16 on the PE, fp32 psum):
  - valid rows packed contiguously, G=32-aligned per set: every set's rows are
    padded with zero-rows to a multiple of G so that every G-row block belongs
    to exactly one set.  Blocks are dealt to 8 cores (SPMD, identical
    programs).  A zero pad row produces the CONSTANT vector
    g = tanh(tanh(b1)@W2+b2) after the two layers; the host subtracts
    n_pad(set) * g, so no selection mask is needed on device.
  - L1: a0 = words^T [e on partitions, rows free]; ps1[h,r] accumulated over
    4 e-chunks; a1 = tanh(ps1 + b1) via per-partition activation bias.
  - L2 TRANSPOSED: ps2[h,r] = sum_h' W2[h',h] a1[h',r] keeps h on partitions,
    so b2 also rides the activation bias (no vector add) and the segment sum
    is a free-dim reduction: VectorE block-reduces a2[h, r] in G-row blocks
    -> acc[h, block].  Host maps blocks to sets.
  - PE does ONLY the two 512x512 GEMMs: 32*R cycles/core @2.4GHz.
  - Startup: DVE memsets a warmup tile early; ~16 dependency-free matmuls keep
    the PE busy from ~5us so the HAM clock gate (4/8 -> 8/8 duty) opens before
    the real data lands; DMAs are issued critical-path-first.
"""

import sys

if "/opt/trn_rl_repo" not in sys.path:
    sys.path.insert(0, "/opt/trn_rl_repo")

import ml_dtypes
import numpy as np

import concourse.bass as bass
import concourse.mybir as mybir
import concourse.tile as tile
from concourse import bacc
from concourse.bass_utils import run_bass_kernel_spmd

B, S, E = 64, 1024, 512
H = 512
NCORES = 8
P = 128
KC = E // P  # 4 contraction chunks
RT = 512     # rows per row-tile (matmul moving dim)
G = 32       # segment alignment granularity (block reduce size)
NBT = RT // G  # blocks per full row tile
N_WARMUP = 12  # bridge until first data lands; slight overshoot keeps the PE
               # continuously busy so the HAM clock gate opens before real work

f32 = mybir.dt.float32
bf16 = mybir.dt.bfloat16

_cache: dict = {}


def _tiles_of(R: int):
    """Row tiles: small 128/256 lead-in (compute starts while the DMA pipe
    is still ramping), then full 512s, remainder last."""
    assert R % G == 0
    if R >= 896:
        body = R - 384
        tl = [128, 256] + [RT] * (body // RT)
        if body % RT:
            tl.append(body % RT)
    else:
        tl = [RT] * (R // RT)
        if R % RT:
            tl.append(R % RT)
    return tl


def _build(R: int):
    if R in _cache:
        return _cache[R]

    tiles = _tiles_of(R)
    nt = len(tiles)
    offs = [sum(tiles[:i]) for i in range(nt)]

    nc = bacc.Bacc("TRN2", target_bir_lowering=False, debug=False, num_devices=NCORES)

    wT_d = nc.dram_tensor("wT", [P, KC, R], bf16, kind="ExternalInput").ap()
    w1_d = nc.dram_tensor("w1", [E, H], bf16, kind="ExternalInput").ap()
    w2_d = nc.dram_tensor("w2", [H, H], bf16, kind="ExternalInput").ap()
    # b1 and b2 packed host-side as [P, 2, KC] (one DMA; sem pool is scarce)
    b12_d = nc.dram_tensor("b12", [P, 2, KC], f32, kind="ExternalInput").ap()
    acc_d = nc.dram_tensor("acc", [P, nt, KC, NBT], f32, kind="ExternalOutput").ap()

    with tile.TileContext(nc) as tc:
        with (
            tc.tile_pool(name="const", bufs=1) as cpool,
            tc.tile_pool(name="a0", bufs=2) as a0pool,
            tc.tile_pool(name="a1", bufs=2) as a1pool,
            tc.tile_pool(name="a2", bufs=2) as a2pool,
            tc.tile_pool(name="psA", bufs=1, space="PSUM") as ps1apool,
            tc.tile_pool(name="psB", bufs=1, space="PSUM") as ps1bpool,
            tc.tile_pool(name="ps2", bufs=4, space="PSUM") as ps2pool,
        ):
            # Dependency tracking is per-TENSOR (coarse), so anything that
            # must overlap needs its own tile: L1 uses two fresh 2-bank
            # half-tiles per row tile (tanh merges a pair of h-chunks in one
            # bias-free activation), L2 uses four 1-bank tiles.

            # PE warmup: DVE memsets the tile early (vector's iram load ends
            # ~4.9us); dependency-free bf16 matmuls keep the PE busy so the
            # HAM clock gate (4/8 duty default) opens before real data lands.
            # All warmups accumulate into ONE psum region: same-engine in-order
            # accumulation needs no semaphores, so they run back-to-back.
            warm_sb = cpool.tile([P, 256], bf16)
            nc.vector.memset(warm_sb[:], 0.25)
            wps = ps2pool.tile([P, RT], f32, tag="ps2", name="wps")
            for w in range(N_WARMUP):
                nc.tensor.matmul(
                    wps[:, :256], warm_sb[:, :P], warm_sb[:, :256],
                    start=(w == 0), stop=(w == N_WARMUP - 1),
                )
            # persistent output accumulator, shipped in ONE contiguous DMA at
            # the end (per-tile strided out-DMAs produce tiny packets).
            # memset covers the unwritten tail-block columns of the last tile.
            acc_all = cpool.tile([P, nt, KC, NBT], f32)
            nc.vector.memset(acc_all[:], 0.0)

            # --- DMA issue: strict priority order matched to consumption.
            # The DMA rings fair-share ~376 GB/s over everything in flight,
            # so the critical tile-0 set (a0[t0]+w1) goes out first, then w2,
            # then later tiles.  Issues alternate sync/scalar (two HWDGE
            # engines) for issue-rate; the shared semaphore pool caps
            # in-flight DMAs at ~8, hence merged transfers.
            a0_pre: dict = {}
            a0c = a0pool.tile([P, KC, RT], bf16, tag="a0", name="a0t0")
            nc.sync.dma_start(a0c[:, :, :tiles[0]], wT_d[:, :, 0:tiles[0]])
            w1sb = cpool.tile([P, KC, H], bf16)
            w1r = w1_d.rearrange("(k p) h -> p k h", p=P)
            nc.scalar.dma_start(w1sb[:, :2, :], w1r[:, :2, :])
            a0_pre[0] = a0c
            if nt > 1:
                a0n = a0pool.tile([P, KC, RT], bf16, tag="a0", name="a0t1")
                nc.sync.dma_start(
                    a0n[:, :, :tiles[1]], wT_d[:, :, offs[1]:offs[1] + tiles[1]]
                )
                a0_pre[1] = a0n
            nc.scalar.dma_start(w1sb[:, 2:, :], w1r[:, 2:, :])
            b12sb = cpool.tile([P, 2, KC], f32)
            nc.scalar.dma_start(b12sb[:], b12_d)
            b1sb = b12sb[:, 0, :]
            b2sb = b12sb[:, 1, :]
            w2sb = cpool.tile([P, KC, H], bf16)
            nc.sync.dma_start(w2sb[:], w2_d.rearrange("(k p) h -> p k h", p=P))

            # --- software-pipelined emission: L1(t+1) before L2(t).  Every
            # a1 chunk is then produced a full 3.4us PE-block before L2 reads
            # it, so the (saturated) scalar engine is never on the critical
            # path.  ScalarE is relieved further by moving the b1 adds to
            # VectorE (tensor_scalar on psum in-place) so the two a1-tanh
            # pairs merge into two bias-free 2-bank activations.
            def emit_l1(t, a1t, k_outer=False):
                nr = tiles[t]
                a0 = a0_pre.pop(t)
                phs = [
                    pool.tile(
                        [P, 2, RT], f32, tag="ps1h",
                        name=f"ps1{'ab'[half]}{t}",
                    )
                    for half, pool in ((0, ps1apool), (1, ps1bpool))
                ]
                # all of a half's matmuls are emitted BEFORE its bias-adds:
                # with coarse per-tensor deps anything else chains the second
                # m-group behind the first bias.  k_outer (tile 0) consumes
                # the two a0/w1 DMA halves in arrival order.
                if k_outer:
                    for k in range(KC):
                        for m in range(KC):
                            nc.tensor.matmul(
                                phs[m // 2][:, m % 2, :nr],
                                w1sb[:, k, m * P:(m + 1) * P],
                                a0[:, k, :nr],
                                start=(k == 0),
                                stop=(k == KC - 1),
                            )
                else:
                    for m in range(KC):
                        for k in range(KC):
                            nc.tensor.matmul(
                                phs[m // 2][:, m % 2, :nr],
                                w1sb[:, k, m * P:(m + 1) * P],
                                a0[:, k, :nr],
                                start=(k == 0),
                                stop=(k == KC - 1),
                            )
                        if m % 2:
                            emit_l1_tail(t, a1t, m // 2, phs[m // 2], nr)
                if k_outer:
                    for half in (0, 1):
                        emit_l1_tail(t, a1t, half, phs[half], nr)

            def emit_l1_tail(t, a1t, half, ph, nr):
                for mi in (0, 1):
                    m = 2 * half + mi
                    nc.vector.tensor_scalar_add(
                        ph[:, mi, :nr], ph[:, mi, :nr], b1sb[:, m:m + 1]
                    )
                nc.scalar.activation(
                    a1t[:, 2 * half:2 * half + 2, :nr], ph[:, :, :nr],
                    mybir.ActivationFunctionType.Tanh,
                )

            def emit_l2(t, a1t):
                nr = tiles[t]
                # k-outer for t==0 (consumes a1 halves in arrival order);
                # m2-outer steady state
                p2 = [
                    ps2pool.tile([P, RT], f32, tag="ps2", name=f"ps2_{m2}")
                    for m2 in range(KC)
                ]
                for m2 in range(KC):
                    for k in range(KC):
                        nc.tensor.matmul(
                            p2[m2][:, :nr],
                            w2sb[:, k, m2 * P:(m2 + 1) * P],
                            a1t[:, k, :nr],
                            start=(k == 0),
                            stop=(k == KC - 1),
                        )
                a2s = []
                for m2 in range(KC):
                    a2 = a2pool.tile([P, RT], bf16, tag=f"a2c{m2}")
                    nc.scalar.activation(
                        a2[:, :nr],
                        p2[m2][:, :nr],
                        mybir.ActivationFunctionType.Tanh,
                        bias=b2sb[:, m2:m2 + 1],
                    )
                    a2s.append((m2, a2))
                return a2s

            def emit_reduce(t, a2s):
                # segment block sums: [h, nb, G] -> [h, nb] on VectorE.
                # Emitted one iteration AFTER the tile's L2 so they land in
                # VectorE's stream behind the next tile's (early, urgent)
                # bias-adds with their inputs already long complete.
                nb = tiles[t] // G
                for m2, a2 in a2s:
                    nc.vector.tensor_reduce(
                        acc_all[:, t, m2, :nb],
                        a2[:, :tiles[t]].rearrange("p (n g) -> p n g", g=G),
                        mybir.AxisListType.X,
                        mybir.AluOpType.add,
                    )

            a1_of = {0: a1pool.tile([P, KC, RT], bf16, tag="a1", name="a1t0")}
            emit_l1(0, a1_of[0])
            pend = None
            for t in range(nt):
                # prefetch a0(t+1): the a0 pool (bufs=2) throttles the issue
                # until L1(t-1) has released its buffer, so startup transfers
                # keep strict priority
                if t + 1 < nt and t + 1 not in a0_pre:
                    a0n = a0pool.tile(
                        [P, KC, RT], bf16, tag="a0", name=f"a0t{t + 1}"
                    )
                    nc.sync.dma_start(
                        a0n[:, :, :tiles[t + 1]],
                        wT_d[:, :, offs[t + 1]:offs[t + 1] + tiles[t + 1]],
                    )
                    a0_pre[t + 1] = a0n
                if t + 1 < nt:
                    a1_of[t + 1] = a1pool.tile(
                        [P, KC, RT], bf16, tag="a1", name=f"a1t{t + 1}"
                    )
                    emit_l1(t + 1, a1_of[t + 1])
                if pend is not None:
                    emit_reduce(t - 1, pend)
                pend = emit_l2(t, a1_of.pop(t))
            emit_reduce(nt - 1, pend)

            # ship tiles [0, nt-1) as soon as they are done; only the last
            # tile's slice rides in the latency-critical tail
            nc.sync.dma_start(acc_d[:, :nt - 1], acc_all[:, :nt - 1])
            nc.sync.dma_start(acc_d[:, nt - 1:], acc_all[:, nt - 1:])

    nc.compile()
    _cache[R] = nc
    return nc


def _pack(words: np.ndarray):
    """Pack valid rows contiguously, G-aligned per set, dealt to 8 cores.

    Returns per-core bf16 wT arrays + global block bookkeeping.
    """
    words = np.asarray(words, dtype=np.float32)
    mask = np.sign(np.abs(words.sum(axis=-1)))  # [B, S], matches reference
    valid = mask > 0
    lengths = valid.sum(axis=1)

    nblk = -(-lengths // G)  # ceil: blocks per set
    total_blocks = int(nblk.sum())
    pcb = -(-total_blocks // NCORES)  # blocks per core
    R = pcb * G

    rows = np.zeros((NCORES * R, E), dtype=np.float32)
    binfo = np.full(NCORES * pcb, -1, dtype=np.int64)  # set id per block
    off = 0
    for b in range(B):
        vb = words[b][valid[b]]
        L = len(vb)
        rows[off:off + L] = vb
        b0 = off // G
        binfo[b0:b0 + nblk[b]] = b
        off += int(nblk[b]) * G

    per_core = []
    for c in range(NCORES):
        chunk = rows[c * R:(c + 1) * R]
        wT = np.ascontiguousarray(
            chunk.T.reshape(KC, P, R).transpose(1, 0, 2)
        ).astype(ml_dtypes.bfloat16)  # [P, KC, R]
        per_core.append(wT)
    return per_core, R, binfo, mask, lengths


def _in_maps(per_core, inputs):
    W1 = np.asarray(inputs["W1"], dtype=ml_dtypes.bfloat16)
    W2 = np.asarray(inputs["W2"], dtype=ml_dtypes.bfloat16)
    b1 = np.asarray(inputs["b1"], dtype=np.float32)
    b2 = np.asarray(inputs["b2"], dtype=np.float32)
    # [P, 2, KC]: b12[p, i, m] = b_i[m*128 + p]
    b12 = np.ascontiguousarray(
        np.stack([b1.reshape(KC, P).T, b2.reshape(KC, P).T], axis=1)
    )
    return [{"wT": wT, "w1": W1, "w2": W2, "b12": b12} for wT in per_core]


def kernel(words, W1, b1, W2, b2, W3, b3, W4, b4, W5, b5, W6, b6):
    per_core, R, binfo, mask, lengths = _pack(words)
    nc = _build(R)
    in_maps = _in_maps(per_core, {"W1": W1, "W2": W2, "b1": b1, "b2": b2})

    res = run_bass_kernel_spmd(nc, in_maps, core_ids=list(range(NCORES)))

    tiles = _tiles_of(R)
    nt = len(tiles)
    pcb = R // G
    hsum = np.zeros((B, H), dtype=np.float32)
    for c in range(NCORES):
        acc = res.results[c]["acc"]  # [P, nt, KC, NBT] f32
        # block vectors in h order (h = m*128 + p)
        bv = np.concatenate(
            [acc[:, t, :, :tiles[t] // G].transpose(2, 1, 0).reshape(-1, H)
             for t in range(nt)], axis=0
        )  # [pcb, H]
        ids = binfo[c * pcb:(c + 1) * pcb]
        sel = ids >= 0
        np.add.at(hsum, ids[sel], bv[sel])

    # exact correction for zero-pad rows: each contributes the constant
    # g = tanh(tanh(b1) @ W2 + b2) (computed with the same bf16 rounding
    # the device uses, in fp32 accumulation)
    b1f = np.asarray(b1, np.float32)
    b2f = np.asarray(b2, np.float32)
    W2q = np.asarray(W2, np.float32).astype(ml_dtypes.bfloat16).astype(np.float32)
    h1g = np.tanh(b1f).astype(ml_dtypes.bfloat16).astype(np.float32)
    g = np.tanh(h1g @ W2q + b2f)
    npad = (-(-lengths // G) * G - lengths).astype(np.float32)  # per set
    hsum -= npad[:, None] * g[None, :]

    # host decode (tiny)
    codes = hsum @ np.asarray(W3, np.float32) + (
        lengths.astype(np.float32)[:, None] * np.asarray(b3, np.float32)
    )
    h = np.tanh(codes @ np.asarray(W4, np.float32) + np.asarray(b4, np.float32))
    h = np.tanh(h @ np.asarray(W5, np.float32) + np.asarray(b5, np.float32))
    out = h @ np.asarray(W6, np.float32) + np.asarray(b6, np.float32)
    return out.astype(np.float32)


# revision 44
# speedup vs baseline: 1.2262x; 1.2262x over previous
"""DeepSet kernel for Trainium2 (8 NeuronCores, data-parallel).

Model (reference):
    mask  = sign(|sum_e words|)                  # padding rows are all-zero
    h1    = tanh(words @ W1 + b1)                # [B,S,H]
    h2    = tanh(h1 @ W2 + b2)                   # [B,S,H]
    enc   = h2 @ W3 + b3                         # [B,S,C]
    codes = sum_s enc * mask                     # [B,C]
    out   = (tanh(tanh(codes@W4+b4)@W5+b5)) @ W6 + b6   # [B,T]

Algebraic restructuring: codes = (sum_s mask*h2) @ W3 + N_b * b3, so only the
two big MLP layers run on device; the tiny decode runs on host.

Layout strategy (all bf16 on the PE, fp32 psum):
  - valid rows packed contiguously, G=32-aligned per set: every set's rows are
    padded with zero-rows to a multiple of G so that every G-row block belongs
    to exactly one set.  Blocks are dealt to 8 cores (SPMD, identical
    programs).  A zero pad row produces the CONSTANT vector
    g = tanh(tanh(b1)@W2+b2) after the two layers; the host subtracts
    n_pad(set) * g, so no selection mask is needed on device.
  - L1: a0 = words^T [e on partitions, rows free]; ps1[h,r] accumulated over
    4 e-chunks; a1 = tanh(ps1 + b1) via per-partition activation bias.
  - L2 TRANSPOSED: ps2[h,r] = sum_h' W2[h',h] a1[h',r] keeps h on partitions,
    so b2 also rides the activation bias (no vector add) and the segment sum
    is a free-dim reduction: VectorE block-reduces a2[h, r] in G-row blocks
    -> acc[h, block].  Host maps blocks to sets.
  - PE does ONLY the two 512x512 GEMMs: 32*R cycles/core @2.4GHz.
  - Startup: DVE memsets a warmup tile early; ~16 dependency-free matmuls keep
    the PE busy from ~5us so the HAM clock gate (4/8 -> 8/8 duty) opens before
    the real data lands; DMAs are issued critical-path-first.
"""

import sys

if "/opt/trn_rl_repo" not in sys.path:
    sys.path.insert(0, "/opt/trn_rl_repo")

import ml_dtypes
import numpy as np

import concourse.bass as bass
import concourse.mybir as mybir
import concourse.tile as tile
from concourse import bacc
from concourse.bass_utils import run_bass_kernel_spmd

B, S, E = 64, 1024, 512
H = 512
NCORES = 8
P = 128
KC = E // P  # 4 contraction chunks
RT = 512     # rows per row-tile (matmul moving dim)
G = 32       # segment alignment granularity (block reduce size)
NBT = RT // G  # blocks per full row tile
N_WARMUP = 17  # bridge until first data lands; slight overshoot keeps the PE
               # continuously busy so the HAM clock gate opens before real work

f32 = mybir.dt.float32
bf16 = mybir.dt.bfloat16

_cache: dict = {}


def _tiles_of(R: int):
    assert R % G == 0
    tl = [RT] * (R // RT)
    if R % RT:
        tl.append(R % RT)
    return tl


def _build(R: int):
    if R in _cache:
        return _cache[R]

    tiles = _tiles_of(R)
    nt = len(tiles)
    offs = [sum(tiles[:i]) for i in range(nt)]

    nc = bacc.Bacc("TRN2", target_bir_lowering=False, debug=False, num_devices=NCORES)

    wT_d = nc.dram_tensor("wT", [P, KC, R], bf16, kind="ExternalInput").ap()
    w1_d = nc.dram_tensor("w1", [E, H], bf16, kind="ExternalInput").ap()
    w2_d = nc.dram_tensor("w2", [H, H], bf16, kind="ExternalInput").ap()
    # b1 and b2 packed host-side as [P, 2, KC] (one DMA; sem pool is scarce)
    b12_d = nc.dram_tensor("b12", [P, 2, KC], f32, kind="ExternalInput").ap()
    acc_d = nc.dram_tensor("acc", [P, nt, KC, NBT], f32, kind="ExternalOutput").ap()

    with tile.TileContext(nc) as tc:
        with (
            tc.tile_pool(name="const", bufs=1) as cpool,
            tc.tile_pool(name="a0", bufs=2) as a0pool,
            tc.tile_pool(name="a1", bufs=2) as a1pool,
            tc.tile_pool(name="a2", bufs=2) as a2pool,
            tc.tile_pool(name="psA", bufs=1, space="PSUM") as ps1apool,
            tc.tile_pool(name="psB", bufs=1, space="PSUM") as ps1bpool,
            tc.tile_pool(name="ps2", bufs=4, space="PSUM") as ps2pool,
        ):
            # Dependency tracking is per-TENSOR (coarse), so anything that
            # must overlap needs its own tile: L1 uses two fresh 2-bank
            # half-tiles per row tile (tanh merges a pair of h-chunks in one
            # bias-free activation), L2 uses four 1-bank tiles.

            # PE warmup: DVE memsets the tile early (vector's iram load ends
            # ~4.9us); dependency-free bf16 matmuls keep the PE busy so the
            # HAM clock gate (4/8 duty default) opens before real data lands.
            # All warmups accumulate into ONE psum region: same-engine in-order
            # accumulation needs no semaphores, so they run back-to-back.
            warm_sb = cpool.tile([P, 256], bf16)
            nc.vector.memset(warm_sb[:], 0.25)
            wps = ps2pool.tile([P, RT], f32, tag="ps2", name="wps")
            for w in range(N_WARMUP):
                nc.tensor.matmul(
                    wps[:, :256], warm_sb[:, :P], warm_sb[:, :256],
                    start=(w == 0), stop=(w == N_WARMUP - 1),
                )
            # persistent output accumulator, shipped in ONE contiguous DMA at
            # the end (per-tile strided out-DMAs produce tiny packets).
            # memset covers the unwritten tail-block columns of the last tile.
            acc_all = cpool.tile([P, nt, KC, NBT], f32)
            nc.vector.memset(acc_all[:], 0.0)

            # --- DMA issue: strict priority order matched to consumption.
            # The DMA rings fair-share ~376 GB/s over everything in flight,
            # so the critical tile-0 set (a0[t0]+w1) goes out first, then w2,
            # then later tiles.  Issues alternate sync/scalar (two HWDGE
            # engines) for issue-rate; the shared semaphore pool caps
            # in-flight DMAs at ~8, hence merged transfers.
            a0_pre: dict = {}
            a0c = a0pool.tile([P, KC, RT], bf16, tag="a0", name="a0t0")
            nc.sync.dma_start(a0c[:, :2, :tiles[0]], wT_d[:, :2, 0:tiles[0]])
            w1sb = cpool.tile([P, KC, H], bf16)
            w1r = w1_d.rearrange("(k p) h -> p k h", p=P)
            nc.scalar.dma_start(w1sb[:, :2, :], w1r[:, :2, :])
            nc.sync.dma_start(a0c[:, 2:, :tiles[0]], wT_d[:, 2:, 0:tiles[0]])
            nc.scalar.dma_start(w1sb[:, 2:, :], w1r[:, 2:, :])
            a0_pre[0] = a0c
            if nt > 1:
                a0n = a0pool.tile([P, KC, RT], bf16, tag="a0", name="a0t1")
                nc.sync.dma_start(
                    a0n[:, :, :tiles[1]], wT_d[:, :, offs[1]:offs[1] + tiles[1]]
                )
                a0_pre[1] = a0n
            b12sb = cpool.tile([P, 2, KC], f32)
            nc.scalar.dma_start(b12sb[:], b12_d)
            b1sb = b12sb[:, 0, :]
            b2sb = b12sb[:, 1, :]
            w2sb = cpool.tile([P, KC, H], bf16)
            nc.sync.dma_start(w2sb[:], w2_d.rearrange("(k p) h -> p k h", p=P))

            # --- software-pipelined emission: L1(t+1) before L2(t).  Every
            # a1 chunk is then produced a full 3.4us PE-block before L2 reads
            # it, so the (saturated) scalar engine is never on the critical
            # path.  ScalarE is relieved further by moving the b1 adds to
            # VectorE (tensor_scalar on psum in-place) so the two a1-tanh
            # pairs merge into two bias-free 2-bank activations.
            def emit_l1(t, a1t, k_outer=False):
                nr = tiles[t]
                a0 = a0_pre.pop(t)
                phs = [
                    pool.tile(
                        [P, 2, RT], f32, tag="ps1h",
                        name=f"ps1{'ab'[half]}{t}",
                    )
                    for half, pool in ((0, ps1apool), (1, ps1bpool))
                ]
                # all of a half's matmuls are emitted BEFORE its bias-adds:
                # with coarse per-tensor deps anything else chains the second
                # m-group behind the first bias.  k_outer (tile 0) consumes
                # the two a0/w1 DMA halves in arrival order.
                if k_outer:
                    for k in range(KC):
                        for m in range(KC):
                            nc.tensor.matmul(
                                phs[m // 2][:, m % 2, :nr],
                                w1sb[:, k, m * P:(m + 1) * P],
                                a0[:, k, :nr],
                                start=(k == 0),
                                stop=(k == KC - 1),
                            )
                else:
                    for m in range(KC):
                        for k in range(KC):
                            nc.tensor.matmul(
                                phs[m // 2][:, m % 2, :nr],
                                w1sb[:, k, m * P:(m + 1) * P],
                                a0[:, k, :nr],
                                start=(k == 0),
                                stop=(k == KC - 1),
                            )
                        if m % 2:
                            emit_l1_tail(t, a1t, m // 2, phs[m // 2], nr)
                if k_outer:
                    for half in (0, 1):
                        emit_l1_tail(t, a1t, half, phs[half], nr)

            def emit_l1_tail(t, a1t, half, ph, nr):
                for mi in (0, 1):
                    m = 2 * half + mi
                    nc.vector.tensor_scalar_add(
                        ph[:, mi, :nr], ph[:, mi, :nr], b1sb[:, m:m + 1]
                    )
                nc.scalar.activation(
                    a1t[:, 2 * half:2 * half + 2, :nr], ph[:, :, :nr],
                    mybir.ActivationFunctionType.Tanh,
                )

            def emit_l2(t, a1t):
                nr = tiles[t]
                # k-outer for t==0 (consumes a1 halves in arrival order);
                # m2-outer steady state
                p2 = [
                    ps2pool.tile([P, RT], f32, tag="ps2", name=f"ps2_{m2}")
                    for m2 in range(KC)
                ]
                for m2 in range(KC):
                    for k in range(KC):
                        nc.tensor.matmul(
                            p2[m2][:, :nr],
                            w2sb[:, k, m2 * P:(m2 + 1) * P],
                            a1t[:, k, :nr],
                            start=(k == 0),
                            stop=(k == KC - 1),
                        )
                a2s = []
                for m2 in range(KC):
                    a2 = a2pool.tile([P, RT], bf16, tag=f"a2c{m2}")
                    nc.scalar.activation(
                        a2[:, :nr],
                        p2[m2][:, :nr],
                        mybir.ActivationFunctionType.Tanh,
                        bias=b2sb[:, m2:m2 + 1],
                    )
                    a2s.append((m2, a2))
                return a2s

            def emit_reduce(t, a2s):
                # segment block sums: [h, nb, G] -> [h, nb] on VectorE.
                # Emitted one iteration AFTER the tile's L2 so they land in
                # VectorE's stream behind the next tile's (early, urgent)
                # bias-adds with their inputs already long complete.
                nb = tiles[t] // G
                for m2, a2 in a2s:
                    nc.vector.tensor_reduce(
                        acc_all[:, t, m2, :nb],
                        a2[:, :tiles[t]].rearrange("p (n g) -> p n g", g=G),
                        mybir.AxisListType.X,
                        mybir.AluOpType.add,
                    )

            a1_of = {0: a1pool.tile([P, KC, RT], bf16, tag="a1", name="a1t0")}
            emit_l1(0, a1_of[0])
            pend = None
            for t in range(nt):
                # prefetch a0(t+1): the a0 pool (bufs=2) throttles the issue
                # until L1(t-1) has released its buffer, so startup transfers
                # keep strict priority
                if t + 1 < nt and t + 1 not in a0_pre:
                    a0n = a0pool.tile(
                        [P, KC, RT], bf16, tag="a0", name=f"a0t{t + 1}"
                    )
                    nc.sync.dma_start(
                        a0n[:, :, :tiles[t + 1]],
                        wT_d[:, :, offs[t + 1]:offs[t + 1] + tiles[t + 1]],
                    )
                    a0_pre[t + 1] = a0n
                if t + 1 < nt:
                    a1_of[t + 1] = a1pool.tile(
                        [P, KC, RT], bf16, tag="a1", name=f"a1t{t + 1}"
                    )
                    emit_l1(t + 1, a1_of[t + 1])
                if pend is not None:
                    emit_reduce(t - 1, pend)
                pend = emit_l2(t, a1_of.pop(t))
            emit_reduce(nt - 1, pend)

            # ship tiles [0, nt-1) as soon as they are done; only the last
            # tile's slice rides in the latency-critical tail
            nc.sync.dma_start(acc_d[:, :nt - 1], acc_all[:, :nt - 1])
            nc.sync.dma_start(acc_d[:, nt - 1:], acc_all[:, nt - 1:])

    nc.compile()
    _cache[R] = nc
    return nc


def _pack(words: np.ndarray):
    """Pack valid rows contiguously, G-aligned per set, dealt to 8 cores.

    Returns per-core bf16 wT arrays + global block bookkeeping.
    """
    words = np.asarray(words, dtype=np.float32)
    mask = np.sign(np.abs(words.sum(axis=-1)))  # [B, S], matches reference
    valid = mask > 0
    lengths = valid.sum(axis=1)

    nblk = -(-lengths // G)  # ceil: blocks per set
    total_blocks = int(nblk.sum())
    pcb = -(-total_blocks // NCORES)  # blocks per core
    R = pcb * G

    rows = np.zeros((NCORES * R, E), dtype=np.float32)
    binfo = np.full(NCORES * pcb, -1, dtype=np.int64)  # set id per block
    off = 0
    for b in range(B):
        vb = words[b][valid[b]]
        L = len(vb)
        rows[off:off + L] = vb
        b0 = off // G
        binfo[b0:b0 + nblk[b]] = b
        off += int(nblk[b]) * G

    per_core = []
    for c in range(NCORES):
        chunk = rows[c * R:(c + 1) * R]
        wT = np.ascontiguousarray(
            chunk.T.reshape(KC, P, R).transpose(1, 0, 2)
        ).astype(ml_dtypes.bfloat16)  # [P, KC, R]
        per_core.append(wT)
    return per_core, R, binfo, mask, lengths


def _in_maps(per_core, inputs):
    W1 = np.asarray(inputs["W1"], dtype=ml_dtypes.bfloat16)
    W2 = np.asarray(inputs["W2"], dtype=ml_dtypes.bfloat16)
    b1 = np.asarray(inputs["b1"], dtype=np.float32)
    b2 = np.asarray(inputs["b2"], dtype=np.float32)
    # [P, 2, KC]: b12[p, i, m] = b_i[m*128 + p]
    b12 = np.ascontiguousarray(
        np.stack([b1.reshape(KC, P).T, b2.reshape(KC, P).T], axis=1)
    )
    return [{"wT": wT, "w1": W1, "w2": W2, "b12": b12} for wT in per_core]


def kernel(words, W1, b1, W2, b2, W3, b3, W4, b4, W5, b5, W6, b6):
    per_core, R, binfo, mask, lengths = _pack(words)
    nc = _build(R)
    in_maps = _in_maps(per_core, {"W1": W1, "W2": W2, "b1": b1, "b2": b2})

    res = run_bass_kernel_spmd(nc, in_maps, core_ids=list(range(NCORES)))

    tiles = _tiles_of(R)
    nt = len(tiles)
    pcb = R // G
    hsum = np.zeros((B, H), dtype=np.float32)
    for c in range(NCORES):
        acc = res.results[c]["acc"]  # [P, nt, KC, NBT] f32
        # block vectors in h order (h = m*128 + p)
        bv = np.concatenate(
            [acc[:, t, :, :tiles[t] // G].transpose(2, 1, 0).reshape(-1, H)
             for t in range(nt)], axis=0
        )  # [pcb, H]
        ids = binfo[c * pcb:(c + 1) * pcb]
        sel = ids >= 0
        np.add.at(hsum, ids[sel], bv[sel])

    # exact correction for zero-pad rows: each contributes the constant
    # g = tanh(tanh(b1) @ W2 + b2) (computed with the same bf16 rounding
    # the device uses, in fp32 accumulation)
    b1f = np.asarray(b1, np.float32)
    b2f = np.asarray(b2, np.float32)
    W2q = np.asarray(W2, np.float32).astype(ml_dtypes.bfloat16).astype(np.float32)
    h1g = np.tanh(b1f).astype(ml_dtypes.bfloat16).astype(np.float32)
    g = np.tanh(h1g @ W2q + b2f)
    npad = (-(-lengths // G) * G - lengths).astype(np.float32)  # per set
    hsum -= npad[:, None] * g[None, :]

    # host decode (tiny)
    codes = hsum @ np.asarray(W3, np.float32) + (
        lengths.astype(np.float32)[:, None] * np.asarray(b3, np.float32)
    )
    h = np.tanh(codes @ np.asarray(W4, np.float32) + np.asarray(b4, np.float32))
    h = np.tanh(h @ np.asarray(W5, np.float32) + np.asarray(b5, np.float32))
    out = h @ np.asarray(W6, np.float32) + np.asarray(b6, np.float32)
    return out.astype(np.float32)
